# revision 35
# baseline (speedup 1.0000x reference)
"""KANLinear (no residual) Trainium2 kernel.

out[b,o] = sum_{i,g} B_g(x[b,i]) * W[o,i,g] where B_g are cubic B-spline
bases on a uniform grid (G=5, k=3, range [-1,1] -> 8 bases, knots
t_j = 0.4*j - 2.2).

Closed form used on-device: with U = 2.5*x + 3.5 and z_g = 2 - |U - g|,

    6 * B_g(x) = relu(z_g)^3 - 4*relu(z_g - 1)^3

(exact for the cardinal cubic B-spline everywhere). The 1/6 is folded
into the weights host-side; the 4x is folded into the h2 hinge via an
input pre-scaled by 4^(1/3) (cube homogeneity). Per 128-infeature tile,
custom DVE subdim instructions produce multi-basis hinge-cube planes in
one shot (PageIdx supplies the per-basis shift g), and a bf16 DVE
tensor_sub (2x 16-bit perf mode) combines them: bb = h1 - h2x4.

The matmul runs bf16 x bf16 -> PSUM fp32 (the fp32r moving operand
streamed at ~2 cycles/col on HW; bf16 streams at 1/col, halving
Tensor-engine time) and is a single gap-free 512-instruction stream at
~219 ns effective per [128x128]x[128x512] matmul — the PE floor.
Dummy warm-up matmuls ramp the PE clock during the input DMA latency;
early tiles use fine-grained basis chunks so the stream starts ~2us
after the first x tile lands; output PSUM banks evacuate on two engines
and DMA out on three queues.

Sharding: data-parallel over tokens (4096 -> 512 per core on 8 cores),
spline_weight replicated (bf16, 16 MB); no collectives, host
concatenates the shards.
"""

import numpy as np

N_CORES = 8
B_TOT = 4096
B_SHARD = B_TOT // N_CORES  # 512
IN_F = 1024
OUT_F = 1024
G = 8  # GRID_SIZE + SPLINE_ORDER
I_TILES = IN_F // 128  # 8
M_TILES = B_SHARD // 128  # 4
N_CHUNKS = OUT_F // 512  # 2
# Per-tile page chunking for the hinge/sub DVE ops. Early tiles are
# fine-grained so the first matmuls start ~2us after the x tile lands;
# later tiles use big fused ops (lower per-instruction overhead).
CHUNKS = {0: (1, 2, 2, 2), 1: (2, 2, 4), 2: (2, 2, 4)}  # t0: g0 handled separately
CHUNKS_DEFAULT = (4, 4)
# Dummy matmuls that ramp the PE clock before the real stream. Sized so the
# chain ends ~14us, the earliest start the DVE basis pipeline can sustain
# without supply gaps (starting earlier stalls the stream and costs more
# than it saves).
N_WARM = 16

_CACHE = {}


def _hinge8_op():
    """Register (once) and return the custom DVE subdim op

        out[p, s, k] = cube(relu(imm2 - |in0[p, s, k] - (s0 + s*s1)|))

    i.e. per page s the shift is s0+s*s1 (PageIdx), then t = in0 - shift;
    a = |t|; z = imm2 - a; out = relu(z)^3. Exactly 8 ALU stages on trn2
    (abs is maxx(t, 0-t); PageIdx costs one stage). The s1 page step lets
    the caller pre-scale the whole hinge by c via in0 = c*U, s1 = c,
    imm2 = Z*c (since relu(c*z)^3 = c^3 relu(z)^3)."""
    if "op" in _CACHE:
        return _CACHE["op"]

    from concourse import dve_ops
    from concourse.dve_ops import DveOp
    from concourse.dve_spec import (
        C0,
        C1,
        C2,
        PageIdx,
        Spec,
        Src0,
        Zero,
        lower,
        maxx,
        relu,
        sq,
    )
    from concourse.dve_uop import DveOpSpec

    name = "BSPLINE_HINGE8_PG"

    def _ref(in0, in1, s0, s1, imm2):
        a = np.asarray(in0, np.float32)
        # in0 arrives [P, S, N]; page shift = s0 + s*s1
        s_idx = np.arange(a.shape[1], dtype=np.float32).reshape(1, -1, 1)
        shift = np.asarray(s0, np.float32).reshape(-1, 1, 1) + s_idx * np.asarray(
            s1, np.float32
        ).reshape(-1, 1, 1)
        z = np.float32(imm2) - np.abs(a - shift)
        r = np.maximum(z, np.float32(0.0))
        return (r * r * r).astype(np.float32)

    pg = PageIdx(C0, C1)
    t = Src0 - pg
    a = maxx(t, Zero - t)
    z = C2 - a
    r = relu(z)
    body = sq(r) * r
    spec = Spec(body=body, reference=_ref)

    if name not in dve_ops._SUB_OPCODE_FOR_NAME:
        row = dve_ops._CUSTOM_DVE_ROW_BASE + len(dve_ops.OPS)
        assert row < 0x20
        shas = {}
        for ver in ("v3", "v4"):
            try:
                tmp = DveOpSpec(
                    name=name, opcode=row, uops=lower(spec, ver=ver), rd1_en=False
                )
                shas[ver] = tmp.sha(ver)
            except Exception:
                pass
        op = DveOp(name, spec, subdim=True, uops_sha=shas)
        dve_ops.OPS.append(op)
        dve_ops._SUB_OPCODE_FOR_NAME[name] = row
        dve_ops.CUSTOM_DVE_SPECS[name] = spec
    else:
        op = next(o for o in dve_ops.OPS if o.name == name)

    _CACHE["op"] = op
    return op


def _hinge1_op():
    """Register (once) and return the single-shift custom DVE op

        out = cube(relu(imm2 - |in0*s0 - s1|))

    (the affine is folded into s0/s1 so it reads x directly — used for the
    first tile's single-basis chunks to keep the ACT u/v ops off the
    critical path)."""
    if "op1" in _CACHE:
        return _CACHE["op1"]

    from concourse import dve_ops
    from concourse.dve_ops import DveOp
    from concourse.dve_spec import C0, C1, C2, Spec, Src0, Zero, lower, maxx, relu, sq
    from concourse.dve_uop import DveOpSpec

    name = "BSPLINE_HINGE_CUBE1"

    def _ref(in0, in1, s0, s1, imm2):
        t = np.asarray(in0, np.float32) * np.asarray(s0, np.float32).reshape(
            -1, 1
        ) - np.asarray(s1, np.float32).reshape(-1, 1)
        z = np.float32(imm2) - np.abs(t)
        r = np.maximum(z, np.float32(0.0))
        return (r * r * r).astype(np.float32)

    m = Src0 * C0
    t = m - C1
    a = maxx(t, Zero - t)
    z = C2 - a
    r = relu(z)
    body = sq(r) * r
    spec = Spec(body=body, reference=_ref)

    if name not in dve_ops._SUB_OPCODE_FOR_NAME:
        row = dve_ops._CUSTOM_DVE_ROW_BASE + len(dve_ops.OPS)
        assert row < 0x20
        shas = {}
        for ver in ("v3", "v4"):
            try:
                tmp = DveOpSpec(
                    name=name, opcode=row, uops=lower(spec, ver=ver), rd1_en=False
                )
                shas[ver] = tmp.sha(ver)
            except Exception:
                pass
        op = DveOp(name, spec, subdim=False, uops_sha=shas)
        dve_ops.OPS.append(op)
        dve_ops._SUB_OPCODE_FOR_NAME[name] = row
        dve_ops.CUSTOM_DVE_SPECS[name] = spec
    else:
        op = next(o for o in dve_ops.OPS if o.name == name)

    _CACHE["op1"] = op
    return op


def _build_nc():
    """Build the per-core Bass program (SPMD: identical on all 8 cores)."""
    if "nc" in _CACHE:
        return _CACHE["nc"]

    from concourse import bacc
    import concourse.mybir as mybir
    import concourse.tile as tile

    op = _hinge8_op()
    op1 = _hinge1_op()

    f32 = mybir.dt.float32
    bf16 = mybir.dt.bfloat16

    nc = bacc.Bacc(None, target_bir_lowering=False)

    x_t = nc.declare_dram_parameter("x_t", [IN_F, B_SHARD], f32, isOutput=False)
    w = nc.declare_dram_parameter("w", [G * IN_F, OUT_F], bf16, isOutput=False)
    # output leaves the device as bf16 (host converts to f32): +0.2% rounding
    # error, but halves the tail HBM write
    out = nc.declare_dram_parameter("out", [B_SHARD, OUT_F], bf16, isOutput=True)

    with tile.TileContext(nc) as tc:
        with (
            tc.tile_pool(name="xp", bufs=3) as xp,
            tc.tile_pool(name="up", bufs=2) as up,
            tc.tile_pool(name="hp", bufs=2) as hp,
            tc.tile_pool(name="bp", bufs=2) as bp,
            tc.tile_pool(name="wp", bufs=6) as wp,
            tc.tile_pool(name="jp", bufs=1) as jp,
            tc.tile_pool(name="outp", bufs=4) as outp,
            tc.tile_pool(name="ps", bufs=1, space="PSUM") as ps,
        ):
            psum = [
                [
                    ps.tile([128, 512], f32, tag=f"ps_{m}_{n}", name=f"ps_{m}_{n}")
                    for n in range(N_CHUNKS)
                ]
                for m in range(M_TILES)
            ]

            # PE p-state warm-up: run dummy matmuls on a zeroed tile while
            # the first x tile + bases are still in flight, so the real
            # stream starts at the ramped clock. psum[0][0] is reset by the
            # first real matmul's start=True.
            jt = jp.tile([128, 512], bf16, tag="jt")
            nc.gpsimd.memset(jt[:, :], 0.0)
            for _ in range(N_WARM):
                nc.tensor.matmul(
                    psum[0][0][:, :],
                    jt[:, 0:128],
                    jt[:, :],
                    start=True,
                    stop=True,
                    skip_group_check=True,
                )

            for t in range(I_TILES):
                xt = xp.tile([128, B_SHARD], f32, tag="xt")
                if t == 0:
                    # two back-to-back transfers on the same queue: the
                    # first token-half lands earlier, unblocking g0 below
                    nc.sync.dma_start(out=xt[:, 0:256], in_=x_t[0:128, 0:256])
                    nc.sync.dma_start(out=xt[:, 256:512], in_=x_t[0:128, 256:512])
                else:
                    nc.sync.dma_start(
                        out=xt[:, :], in_=x_t[t * 128 : (t + 1) * 128, :]
                    )

                # U = 2.5*x + 3.5 and V = c*U (c = 4^(1/3)) on the
                # (otherwise idle) Activation engine
                c4 = 4.0 ** (1.0 / 3.0)
                ut = up.tile([128, B_SHARD], f32, tag="ut")
                vt = up.tile([128, B_SHARD], f32, tag="vt")
                nc.scalar.activation(
                    out=ut[:, :],
                    in_=xt[:, :],
                    func=mybir.ActivationFunctionType.Copy,
                    bias=3.5,
                    scale=2.5,
                )
                nc.scalar.activation(
                    out=vt[:, :],
                    in_=xt[:, :],
                    func=mybir.ActivationFunctionType.Copy,
                    bias=3.5 * c4,
                    scale=2.5 * c4,
                )

                h1 = hp.tile([128, G, B_SHARD], bf16, tag="h1")
                h2 = hp.tile([128, G, B_SHARD], bf16, tag="h2")
                bb = bp.tile([128, G, B_SHARD], bf16, tag="bb")
                g0 = 0
                if t == 0:
                    # g0 by token-halves so the first matmuls (which only
                    # need tokens 0:256) start as soon as the first x half
                    # lands
                    for half in (slice(0, 256), slice(256, 512)):
                        nc.vector._custom_dve(
                            op1,
                            out=h1[:, 0:1, half],
                            in0=xt[:, half],
                            s0=2.5,
                            s1=-3.5,
                            imm2=2.0,
                        )
                        nc.vector._custom_dve(
                            op1,
                            out=h2[:, 0:1, half],
                            in0=xt[:, half],
                            s0=2.5 * c4,
                            s1=-3.5 * c4,
                            imm2=c4,
                        )
                        nc.vector.tensor_sub(
                            bb[:, 0:1, half], h1[:, 0:1, half], h2[:, 0:1, half]
                        )
                    g0 = 1
                for npg in CHUNKS.get(t, CHUNKS_DEFAULT):
                    sl = slice(g0, g0 + npg)
                    # h1 = relu(2 - |U - g|)^3 ; h2 = 4*relu(1 - |U - g|)^3
                    if npg == 1:
                        # single-basis chunk: read x directly (affine in
                        # s0/s1) so it doesn't wait on the ACT u/v copies
                        nc.vector._custom_dve(
                            op1,
                            out=h1[:, sl, :],
                            in0=xt[:, :],
                            s0=2.5,
                            s1=g0 - 3.5,
                            imm2=2.0,
                        )
                        nc.vector._custom_dve(
                            op1,
                            out=h2[:, sl, :],
                            in0=xt[:, :],
                            s0=2.5 * c4,
                            s1=(g0 - 3.5) * c4,
                            imm2=c4,
                        )
                        nc.vector.tensor_sub(
                            bb[:, sl, :], h1[:, sl, :], h2[:, sl, :]
                        )
                        g0 += npg
                        continue
                    ub = ut[:, :].unsqueeze(1).broadcast_to([128, npg, B_SHARD])
                    vb = vt[:, :].unsqueeze(1).broadcast_to([128, npg, B_SHARD])
                    nc.vector._custom_dve(
                        op, out=h1[:, sl, :], in0=ub, s0=float(g0), s1=1.0, imm2=2.0
                    )
                    nc.vector._custom_dve(
                        op,
                        out=h2[:, sl, :],
                        in0=vb,
                        s0=float(g0) * c4,
                        s1=c4,
                        imm2=c4,
                    )
                    # bb = h1 - 4*h2  (= 6*B; the 1/6 lives in the weights).
                    # All-bf16 operands -> the standard tensor_tensor runs in
                    # the DVE 2x/4x 16-bit perf mode.
                    nc.vector.tensor_sub(bb[:, sl, :], h1[:, sl, :], h2[:, sl, :])
                    g0 += npg

                # last tile: keep the final 4 K-chunks out of this loop so
                # they can be m-staggered below
                n_inline = G if t < I_TILES - 1 else G - 4
                for g in range(n_inline):
                    wt = wp.tile([128, OUT_F], bf16, tag="wt")
                    r0 = g * IN_F + t * 128
                    nc.sync.dma_start(out=wt[:, :], in_=w[r0 : r0 + 128, :])
                    first = t == 0 and g == 0
                    for m in range(M_TILES):
                        lhsT = bb[:, g, m * 128 : (m + 1) * 128]
                        for n in range(N_CHUNKS):
                            nc.tensor.matmul(
                                psum[m][n][:, :],
                                lhsT,
                                wt[:, n * 512 : (n + 1) * 512],
                                start=first,
                                stop=False,
                            )

            # Final 4 K-chunks m-staggered: m0 finishes its accumulation
            # ~5us before m3, so its PSUM evac + output DMA overlap the
            # remaining matmuls instead of serializing after the stream.
            # Output DMAs avoid the gpsimd queue (its transfers run ~3x
            # slower than sync/scalar).
            tl = I_TILES - 1
            wts = []
            for g in range(G - 4, G):
                wt = wp.tile([128, OUT_F], bf16, tag="wt")
                r0 = g * IN_F + tl * 128
                nc.sync.dma_start(out=wt[:, :], in_=w[r0 : r0 + 128, :])
                wts.append(wt)
            # Per-queue DMA rate is ~146GB/s (gpsimd ~93), so schedule the
            # 2MB out-write to finish ~1us after the last evac: early m's
            # go whole on one queue, late m's as halves on parallel queues.
            out_plan = {
                0: [(slice(0, 1024), nc.sync)],
                1: [(slice(0, 1024), nc.scalar)],
                2: [(slice(0, 512), nc.gpsimd), (slice(512, 1024), nc.sync)],
                # m3 is the last transfer standing after the stream: quarters
                # on alternating queues shorten its critical tail
                3: [
                    (slice(0, 256), nc.sync),
                    (slice(256, 512), nc.scalar),
                    (slice(512, 768), nc.sync),
                    (slice(768, 1024), nc.scalar),
                ],
            }
            for m in range(M_TILES):
                for gi, g in enumerate(range(G - 4, G)):
                    lhsT = bb[:, g, m * 128 : (m + 1) * 128]
                    for n in range(N_CHUNKS):
                        nc.tensor.matmul(
                            psum[m][n][:, :],
                            lhsT,
                            wts[gi][:, n * 512 : (n + 1) * 512],
                            start=False,
                            stop=g == G - 1,
                        )
                ot = outp.tile([128, OUT_F], bf16, tag="ot")
                rows = slice(m * 128, (m + 1) * 128)
                nc.scalar.copy(out=ot[:, 0:512], in_=psum[m][0][:, :])
                nc.vector.tensor_copy(ot[:, 512:1024], psum[m][1][:, :])
                for cols, q in out_plan[m]:
                    q.dma_start(out=out[rows, cols], in_=ot[:, cols])

    nc.finalize()
    _CACHE["nc"] = nc
    return nc


def _in_maps(x, w2):
    maps = []
    for c in range(N_CORES):
        xs = x[c * B_SHARD : (c + 1) * B_SHARD, :]
        maps.append({"x_t": np.ascontiguousarray(xs.T), "w": w2})
    return maps


def kernel(x, spline_weight, _trace=False):
    import ml_dtypes

    x = np.ascontiguousarray(np.asarray(x, dtype=np.float32))
    W = np.asarray(spline_weight, dtype=np.float32)
    assert x.shape == (B_TOT, IN_F) and W.shape == (OUT_F, IN_F, G)

    # w2[g*IN_F + i, o] = W[o, i, g] / 6  (bf16; the on-device bases are 6*B)
    w2 = np.ascontiguousarray(
        (W.transpose(2, 1, 0) / 6.0).reshape(G * IN_F, OUT_F).astype(ml_dtypes.bfloat16)
    )

    from concourse.bass_utils import run_bass_kernel_spmd

    nc = _build_nc()
    res = run_bass_kernel_spmd(nc, _in_maps(x, w2), list(range(N_CORES)), trace=_trace)
    out = np.concatenate(
        [np.asarray(res.results[c]["out"]) for c in range(N_CORES)], axis=0
    )
    if _trace:
        _CACHE["last_result"] = res
    # device output is bf16; the contract is f32
    return out.astype(np.float32)


# revision 36
# speedup vs baseline: 1.0118x; 1.0118x over previous
"""KANLinear (no residual) Trainium2 kernel.

out[b,o] = sum_{i,g} B_g(x[b,i]) * W[o,i,g] where B_g are cubic B-spline
bases on a uniform grid (G=5, k=3, range [-1,1] -> 8 bases, knots
t_j = 0.4*j - 2.2).

Closed form used on-device: with U = 2.5*x + 3.5 and z_g = 2 - |U - g|,

    6 * B_g(x) = relu(z_g)^3 - 4*relu(z_g - 1)^3

(exact for the cardinal cubic B-spline everywhere). The 1/6 is folded
into the weights host-side; the 4x is folded into the h2 hinge via an
input pre-scaled by 4^(1/3) (cube homogeneity). Per 128-infeature tile,
custom DVE subdim instructions produce multi-basis hinge-cube planes in
one shot (PageIdx supplies the per-basis shift g), and a bf16 DVE
tensor_sub (2x 16-bit perf mode) combines them: bb = h1 - h2x4.

The matmul runs bf16 x bf16 -> PSUM fp32 (the fp32r moving operand
streamed at ~2 cycles/col on HW; bf16 streams at 1/col, halving
Tensor-engine time) and is a single gap-free 512-instruction stream at
~219 ns effective per [128x128]x[128x512] matmul — the PE floor.
Dummy warm-up matmuls ramp the PE clock during the input DMA latency;
early tiles use fine-grained basis chunks so the stream starts ~2us
after the first x tile lands; output PSUM banks evacuate on two engines
and DMA out on three queues.

Sharding: data-parallel over tokens (4096 -> 512 per core on 8 cores),
spline_weight replicated (bf16, 16 MB); no collectives, host
concatenates the shards.
"""

import numpy as np

N_CORES = 8
B_TOT = 4096
B_SHARD = B_TOT // N_CORES  # 512
IN_F = 1024
OUT_F = 1024
G = 8  # GRID_SIZE + SPLINE_ORDER
I_TILES = IN_F // 128  # 8
M_TILES = B_SHARD // 128  # 4
N_CHUNKS = OUT_F // 512  # 2
# Per-tile page chunking for the hinge/sub DVE ops. Early tiles are
# fine-grained so the first matmuls start ~2us after the x tile lands;
# later tiles use big fused ops (lower per-instruction overhead).
CHUNKS = {0: (1, 2, 2, 2), 1: (2, 2, 4), 2: (2, 2, 4)}  # t0: g0 handled separately
CHUNKS_DEFAULT = (4, 4)
# Dummy matmuls that ramp the PE clock before the real stream. Sized so the
# chain ends ~14us, the earliest start the DVE basis pipeline can sustain
# without supply gaps (starting earlier stalls the stream and costs more
# than it saves).
N_WARM = 16

_CACHE = {}


def _hinge8_op():
    """Register (once) and return the custom DVE subdim op

        out[p, s, k] = cube(relu(imm2 - |in0[p, s, k] - (s0 + s*s1)|))

    i.e. per page s the shift is s0+s*s1 (PageIdx), then t = in0 - shift;
    a = |t|; z = imm2 - a; out = relu(z)^3. Exactly 8 ALU stages on trn2
    (abs is maxx(t, 0-t); PageIdx costs one stage). The s1 page step lets
    the caller pre-scale the whole hinge by c via in0 = c*U, s1 = c,
    imm2 = Z*c (since relu(c*z)^3 = c^3 relu(z)^3)."""
    if "op" in _CACHE:
        return _CACHE["op"]

    from concourse import dve_ops
    from concourse.dve_ops import DveOp
    from concourse.dve_spec import (
        C0,
        C1,
        C2,
        PageIdx,
        Spec,
        Src0,
        Zero,
        lower,
        maxx,
        relu,
        sq,
    )
    from concourse.dve_uop import DveOpSpec

    name = "BSPLINE_HINGE8_PG"

    def _ref(in0, in1, s0, s1, imm2):
        a = np.asarray(in0, np.float32)
        # in0 arrives [P, S, N]; page shift = s0 + s*s1
        s_idx = np.arange(a.shape[1], dtype=np.float32).reshape(1, -1, 1)
        shift = np.asarray(s0, np.float32).reshape(-1, 1, 1) + s_idx * np.asarray(
            s1, np.float32
        ).reshape(-1, 1, 1)
        z = np.float32(imm2) - np.abs(a - shift)
        r = np.maximum(z, np.float32(0.0))
        return (r * r * r).astype(np.float32)

    pg = PageIdx(C0, C1)
    t = Src0 - pg
    a = maxx(t, Zero - t)
    z = C2 - a
    r = relu(z)
    body = sq(r) * r
    spec = Spec(body=body, reference=_ref)

    if name not in dve_ops._SUB_OPCODE_FOR_NAME:
        row = dve_ops._CUSTOM_DVE_ROW_BASE + len(dve_ops.OPS)
        assert row < 0x20
        shas = {}
        for ver in ("v3", "v4"):
            try:
                tmp = DveOpSpec(
                    name=name, opcode=row, uops=lower(spec, ver=ver), rd1_en=False
                )
                shas[ver] = tmp.sha(ver)
            except Exception:
                pass
        op = DveOp(name, spec, subdim=True, uops_sha=shas)
        dve_ops.OPS.append(op)
        dve_ops._SUB_OPCODE_FOR_NAME[name] = row
        dve_ops.CUSTOM_DVE_SPECS[name] = spec
    else:
        op = next(o for o in dve_ops.OPS if o.name == name)

    _CACHE["op"] = op
    return op


def _hinge1_op():
    """Register (once) and return the single-shift custom DVE op

        out = cube(relu(imm2 - |in0*s0 - s1|))

    (the affine is folded into s0/s1 so it reads x directly — used for the
    first tile's single-basis chunks to keep the ACT u/v ops off the
    critical path)."""
    if "op1" in _CACHE:
        return _CACHE["op1"]

    from concourse import dve_ops
    from concourse.dve_ops import DveOp
    from concourse.dve_spec import C0, C1, C2, Spec, Src0, Zero, lower, maxx, relu, sq
    from concourse.dve_uop import DveOpSpec

    name = "BSPLINE_HINGE_CUBE1"

    def _ref(in0, in1, s0, s1, imm2):
        t = np.asarray(in0, np.float32) * np.asarray(s0, np.float32).reshape(
            -1, 1
        ) - np.asarray(s1, np.float32).reshape(-1, 1)
        z = np.float32(imm2) - np.abs(t)
        r = np.maximum(z, np.float32(0.0))
        return (r * r * r).astype(np.float32)

    m = Src0 * C0
    t = m - C1
    a = maxx(t, Zero - t)
    z = C2 - a
    r = relu(z)
    body = sq(r) * r
    spec = Spec(body=body, reference=_ref)

    if name not in dve_ops._SUB_OPCODE_FOR_NAME:
        row = dve_ops._CUSTOM_DVE_ROW_BASE + len(dve_ops.OPS)
        assert row < 0x20
        shas = {}
        for ver in ("v3", "v4"):
            try:
                tmp = DveOpSpec(
                    name=name, opcode=row, uops=lower(spec, ver=ver), rd1_en=False
                )
                shas[ver] = tmp.sha(ver)
            except Exception:
                pass
        op = DveOp(name, spec, subdim=False, uops_sha=shas)
        dve_ops.OPS.append(op)
        dve_ops._SUB_OPCODE_FOR_NAME[name] = row
        dve_ops.CUSTOM_DVE_SPECS[name] = spec
    else:
        op = next(o for o in dve_ops.OPS if o.name == name)

    _CACHE["op1"] = op
    return op


def _build_nc():
    """Build the per-core Bass program (SPMD: identical on all 8 cores)."""
    if "nc" in _CACHE:
        return _CACHE["nc"]

    from concourse import bacc
    import concourse.mybir as mybir
    import concourse.tile as tile

    op = _hinge8_op()
    op1 = _hinge1_op()

    f32 = mybir.dt.float32
    bf16 = mybir.dt.bfloat16

    nc = bacc.Bacc(None, target_bir_lowering=False)

    x_t = nc.declare_dram_parameter("x_t", [IN_F, B_SHARD], f32, isOutput=False)
    w = nc.declare_dram_parameter("w", [G * IN_F, OUT_F], bf16, isOutput=False)
    # output leaves the device as bf16 (host converts to f32): +0.2% rounding
    # error, but halves the tail HBM write
    out = nc.declare_dram_parameter("out", [B_SHARD, OUT_F], bf16, isOutput=True)

    with tile.TileContext(nc) as tc:
        with (
            tc.tile_pool(name="xp", bufs=3) as xp,
            tc.tile_pool(name="up", bufs=2) as up,
            tc.tile_pool(name="hp", bufs=2) as hp,
            tc.tile_pool(name="bp", bufs=2) as bp,
            tc.tile_pool(name="wp", bufs=6) as wp,
            tc.tile_pool(name="jp", bufs=1) as jp,
            tc.tile_pool(name="outp", bufs=4) as outp,
            tc.tile_pool(name="ps", bufs=1, space="PSUM") as ps,
        ):
            psum = [
                [
                    ps.tile([128, 512], f32, tag=f"ps_{m}_{n}", name=f"ps_{m}_{n}")
                    for n in range(N_CHUNKS)
                ]
                for m in range(M_TILES)
            ]

            # PE p-state warm-up: run dummy matmuls on a zeroed tile while
            # the first x tile + bases are still in flight, so the real
            # stream starts at the ramped clock. psum[0][0] is reset by the
            # first real matmul's start=True.
            jt = jp.tile([128, 512], bf16, tag="jt")
            nc.gpsimd.memset(jt[:, :], 0.0)
            for _ in range(N_WARM):
                nc.tensor.matmul(
                    psum[0][0][:, :],
                    jt[:, 0:128],
                    jt[:, :],
                    start=True,
                    stop=True,
                    skip_group_check=True,
                )

            for t in range(I_TILES):
                xt = xp.tile([128, B_SHARD], f32, tag="xt")
                if t == 0:
                    # two back-to-back transfers on the same queue: the
                    # first token-half lands earlier, unblocking g0 below
                    nc.sync.dma_start(out=xt[:, 0:256], in_=x_t[0:128, 0:256])
                    nc.sync.dma_start(out=xt[:, 256:512], in_=x_t[0:128, 256:512])
                else:
                    nc.sync.dma_start(
                        out=xt[:, :], in_=x_t[t * 128 : (t + 1) * 128, :]
                    )

                # U = 2.5*x + 3.5 and V = c*U (c = 4^(1/3)) on the
                # (otherwise idle) Activation engine
                c4 = 4.0 ** (1.0 / 3.0)
                ut = up.tile([128, B_SHARD], f32, tag="ut")
                vt = up.tile([128, B_SHARD], f32, tag="vt")
                nc.scalar.activation(
                    out=ut[:, :],
                    in_=xt[:, :],
                    func=mybir.ActivationFunctionType.Copy,
                    bias=3.5,
                    scale=2.5,
                )
                nc.scalar.activation(
                    out=vt[:, :],
                    in_=xt[:, :],
                    func=mybir.ActivationFunctionType.Copy,
                    bias=3.5 * c4,
                    scale=2.5 * c4,
                )

                h1 = hp.tile([128, G, B_SHARD], bf16, tag="h1")
                h2 = hp.tile([128, G, B_SHARD], bf16, tag="h2")
                bb = bp.tile([128, G, B_SHARD], bf16, tag="bb")
                g0 = 0
                if t == 0:
                    # g0 by token-halves so the first matmuls (which only
                    # need tokens 0:256) start as soon as the first x half
                    # lands
                    for half in (slice(0, 256), slice(256, 512)):
                        nc.vector._custom_dve(
                            op1,
                            out=h1[:, 0:1, half],
                            in0=xt[:, half],
                            s0=2.5,
                            s1=-3.5,
                            imm2=2.0,
                        )
                        nc.vector._custom_dve(
                            op1,
                            out=h2[:, 0:1, half],
                            in0=xt[:, half],
                            s0=2.5 * c4,
                            s1=-3.5 * c4,
                            imm2=c4,
                        )
                        nc.vector.tensor_sub(
                            bb[:, 0:1, half], h1[:, 0:1, half], h2[:, 0:1, half]
                        )
                    g0 = 1
                for npg in CHUNKS.get(t, CHUNKS_DEFAULT):
                    sl = slice(g0, g0 + npg)
                    # h1 = relu(2 - |U - g|)^3 ; h2 = 4*relu(1 - |U - g|)^3
                    if npg == 1:
                        # single-basis chunk: read x directly (affine in
                        # s0/s1) so it doesn't wait on the ACT u/v copies
                        nc.vector._custom_dve(
                            op1,
                            out=h1[:, sl, :],
                            in0=xt[:, :],
                            s0=2.5,
                            s1=g0 - 3.5,
                            imm2=2.0,
                        )
                        nc.vector._custom_dve(
                            op1,
                            out=h2[:, sl, :],
                            in0=xt[:, :],
                            s0=2.5 * c4,
                            s1=(g0 - 3.5) * c4,
                            imm2=c4,
                        )
                        nc.vector.tensor_sub(
                            bb[:, sl, :], h1[:, sl, :], h2[:, sl, :]
                        )
                        g0 += npg
                        continue
                    ub = ut[:, :].unsqueeze(1).broadcast_to([128, npg, B_SHARD])
                    vb = vt[:, :].unsqueeze(1).broadcast_to([128, npg, B_SHARD])
                    nc.vector._custom_dve(
                        op, out=h1[:, sl, :], in0=ub, s0=float(g0), s1=1.0, imm2=2.0
                    )
                    nc.vector._custom_dve(
                        op,
                        out=h2[:, sl, :],
                        in0=vb,
                        s0=float(g0) * c4,
                        s1=c4,
                        imm2=c4,
                    )
                    # bb = h1 - 4*h2  (= 6*B; the 1/6 lives in the weights).
                    # All-bf16 operands -> the standard tensor_tensor runs in
                    # the DVE 2x/4x 16-bit perf mode.
                    nc.vector.tensor_sub(bb[:, sl, :], h1[:, sl, :], h2[:, sl, :])
                    g0 += npg

                # last tile: keep the final 4 K-chunks out of this loop so
                # they can be m-staggered below
                n_inline = G if t < I_TILES - 1 else G - 4
                for g in range(n_inline):
                    wt = wp.tile([128, OUT_F], bf16, tag="wt")
                    r0 = g * IN_F + t * 128
                    nc.sync.dma_start(out=wt[:, :], in_=w[r0 : r0 + 128, :])
                    first = t == 0 and g == 0
                    for m in range(M_TILES):
                        lhsT = bb[:, g, m * 128 : (m + 1) * 128]
                        for n in range(N_CHUNKS):
                            nc.tensor.matmul(
                                psum[m][n][:, :],
                                lhsT,
                                wt[:, n * 512 : (n + 1) * 512],
                                start=first,
                                stop=False,
                            )

            # Final 4 K-chunks m-staggered: m0 finishes its accumulation
            # ~5us before m3, so its PSUM evac + output DMA overlap the
            # remaining matmuls instead of serializing after the stream.
            # Output DMAs avoid the gpsimd queue (its transfers run ~3x
            # slower than sync/scalar).
            tl = I_TILES - 1
            wts = []
            for g in range(G - 4, G):
                wt = wp.tile([128, OUT_F], bf16, tag="wt")
                r0 = g * IN_F + tl * 128
                nc.sync.dma_start(out=wt[:, :], in_=w[r0 : r0 + 128, :])
                wts.append(wt)
            # Per-queue DMA rate is ~146GB/s (gpsimd ~93), so schedule the
            # 2MB out-write to finish ~1us after the last evac: early m's
            # go whole on one queue, late m's as halves on parallel queues.
            out_plan = {
                0: [(slice(0, 1024), nc.sync)],
                1: [(slice(0, 1024), nc.scalar)],
                2: [(slice(0, 512), nc.gpsimd), (slice(512, 1024), nc.sync)],
                3: [(slice(0, 512), nc.sync), (slice(512, 1024), nc.scalar)],
            }
            for m in range(M_TILES):
                for gi, g in enumerate(range(G - 4, G)):
                    lhsT = bb[:, g, m * 128 : (m + 1) * 128]
                    for n in range(N_CHUNKS):
                        nc.tensor.matmul(
                            psum[m][n][:, :],
                            lhsT,
                            wts[gi][:, n * 512 : (n + 1) * 512],
                            start=False,
                            stop=g == G - 1,
                        )
                ot = outp.tile([128, OUT_F], bf16, tag="ot")
                rows = slice(m * 128, (m + 1) * 128)
                nc.scalar.copy(out=ot[:, 0:512], in_=psum[m][0][:, :])
                nc.vector.tensor_copy(ot[:, 512:1024], psum[m][1][:, :])
                for cols, q in out_plan[m]:
                    q.dma_start(out=out[rows, cols], in_=ot[:, cols])

    nc.finalize()
    _CACHE["nc"] = nc
    return nc


def _in_maps(x, w2):
    maps = []
    for c in range(N_CORES):
        xs = x[c * B_SHARD : (c + 1) * B_SHARD, :]
        maps.append({"x_t": np.ascontiguousarray(xs.T), "w": w2})
    return maps


def kernel(x, spline_weight, _trace=False):
    import ml_dtypes

    x = np.ascontiguousarray(np.asarray(x, dtype=np.float32))
    W = np.asarray(spline_weight, dtype=np.float32)
    assert x.shape == (B_TOT, IN_F) and W.shape == (OUT_F, IN_F, G)

    # w2[g*IN_F + i, o] = W[o, i, g] / 6  (bf16; the on-device bases are 6*B)
    w2 = np.ascontiguousarray(
        (W.transpose(2, 1, 0) / 6.0).reshape(G * IN_F, OUT_F).astype(ml_dtypes.bfloat16)
    )

    from concourse.bass_utils import run_bass_kernel_spmd

    nc = _build_nc()
    res = run_bass_kernel_spmd(nc, _in_maps(x, w2), list(range(N_CORES)), trace=_trace)
    out = np.concatenate(
        [np.asarray(res.results[c]["out"]) for c in range(N_CORES)], axis=0
    )
    if _trace:
        _CACHE["last_result"] = res
    # device output is bf16; the contract is f32
    return out.astype(np.float32)
